# revision 23
# baseline (speedup 1.0000x reference)
"""GraphTransformer (2x GCNConv + global MHA) on 8 TRN2 NeuronCores.

Strategy
--------
Nodes (N=4096) are sharded 512/core. The GCN scatter-add is reformulated as a
dense SpMM: the host builds the normalized adjacency-with-self-loops matrix
A[dst, src] = sum(norm) once from edge_index (pure index preprocessing; every
FLOP of the network itself runs on device), and each core holds its 512-row
slice of A, transposed, as the matmul stationary operand.

Per core (rows R_c):
  conv1 is reassociated as (A @ X) @ W1 — the aggregation runs in the 256-dim
  input space (axT = X^T-nodemajor @ A_c^T, 2 m-tiles) and the W1 transform is
  applied to the 512-row slice only. This avoids both the replicated X@W1 and
  the 512-dim-space aggregation.
  H2p_c  = h1_c @ W2                 -> AllGather over cores -> H2p (full)
  h2_c^T = relu(H2p^T @ A_c^T + b2)  (conv2 aggregation)
  xg_c   = h2_c @ lin_w
  MHA: qT/kT/v from X^T (K,V replicated-computed, attention global over nodes),
       scores computed transposed sT=[keys, queries] per head, exp on ACT
       (logits are O(1) so no max-subtraction is needed), denominator obtained
       by appending a ones-column to V in the attn@V matmul. The k-bias is
       dropped (softmax-invariant) and the v-bias is folded into the output
       row bias on the host. All 4 heads' denominators are copied into one
       [4, 512] tile so the reciprocal runs 4 lanes wide, then broadcast via
       a K=1 bf16 matmul.
  out_c  = relu(xg_c + o_c @ (opw^T @ pw) + row_bias)

All matmuls run in bf16 with fp32 PSUM accumulation. The attention phase is
emitted before conv2 so it overlaps the AllGather.
"""

import os
import sys

import numpy as np
import ml_dtypes

try:
    import concourse  # noqa: F401
except ImportError:  # pragma: no cover
    sys.path.insert(0, "/opt/trn_rl_repo")

from concourse import bacc, bass, mybir, tile
from concourse.bass_utils import run_bass_kernel_spmd

P = 128
N_NODES = 4096
E_EDGES = 131072
IN_DIM = 256
HID = 512
CLS = 256
HEADS = 4
HDIM = 64
NC = 8
RPC = N_NODES // NC  # 512 rows per core

BF = mybir.dt.bfloat16
F32 = mybir.dt.float32
AF = mybir.ActivationFunctionType

KCH_IN = IN_DIM // P    # 2
KCH_HID = HID // P      # 4
NT = N_NODES // P       # 32 node tiles
MT_Q = RPC // P         # 4 query tiles per core


def _emit(tc):
    nc = tc.nc

    # ---------------- I/O ----------------
    xn = nc.dram_tensor("xn", [N_NODES, IN_DIM], BF, kind="ExternalInput")
    xT = nc.dram_tensor("xT", [IN_DIM, N_NODES], BF, kind="ExternalInput")
    xq = nc.dram_tensor("xq", [IN_DIM, RPC], BF, kind="ExternalInput")
    aT = nc.dram_tensor("aT", [N_NODES, RPC], BF, kind="ExternalInput")
    w1 = nc.dram_tensor("w1", [IN_DIM, HID], BF, kind="ExternalInput")
    w2 = nc.dram_tensor("w2", [HID, HID], BF, kind="ExternalInput")
    lw = nc.dram_tensor("lw", [HID, CLS], BF, kind="ExternalInput")
    ipT = nc.dram_tensor("ipT", [IN_DIM, 3 * IN_DIM], BF, kind="ExternalInput")
    opw = nc.dram_tensor("opw", [IN_DIM, IN_DIM], BF, kind="ExternalInput")
    pw = nc.dram_tensor("pw", [IN_DIM, CLS], BF, kind="ExternalInput")
    bp = nc.dram_tensor("bp", [P, 10], F32, kind="ExternalInput")
    brow = nc.dram_tensor("brow", [1, CLS], BF, kind="ExternalInput")
    out = nc.dram_tensor("out", [RPC, CLS], F32, kind="ExternalOutput")

    from contextlib import ExitStack

    with ExitStack() as ctx:
        sb = ctx.enter_context(tc.tile_pool(name="sb", bufs=1))
        pp = ctx.enter_context(tc.tile_pool(name="pp", bufs=1, space="PSUM"))
        dr = ctx.enter_context(tc.tile_pool(name="dr", bufs=1, space="DRAM"))

        # ---------------- input DMA (issued in consumer order) -------------
        # Descriptor issue costs ~0.6-1.4us on the issuing sequencer, so
        # inputs are packed into few large descriptors (2-4 row-chunks per
        # DMA, rearranged into [P, c, d] tiles) and spread over sync+scalar.
        # gpsimd is kept empty so the AllGather triggers (gpsimd-only by NRT
        # contract) fire the moment their input data is ready.
        w1a = sb.tile([P, KCH_IN, HID], BF, name="w1a", tag="w1a")
        nc.scalar.dma_start(out=w1a, in_=w1.rearrange("(c p) d -> p c d", p=P))
        bps = sb.tile([P, 10], F32, name="bps", tag="bps")
        nc.scalar.dma_start(out=bps, in_=bp[:, :])
        xqa = sb.tile([P, KCH_IN, RPC], BF, name="xqa", tag="xqa")
        nc.scalar.dma_start(out=xqa, in_=xq.rearrange("(c p) d -> p c d", p=P))
        ipTs = []
        for k in range(KCH_IN):
            t = sb.tile([P, 3 * IN_DIM], BF, name=f"ipTs{k}", tag=f"ipTs{k}")
            nc.scalar.dma_start(out=t, in_=ipT[k * P:(k + 1) * P, :])
            ipTs.append(t)
        xn2, aT2 = [], []
        for g in range(NT // 2):
            qa, qb = (nc.sync, nc.scalar) if g % 2 == 0 else (nc.scalar, nc.sync)
            t = sb.tile([P, 2, IN_DIM], BF, name=f"xn2_{g}", tag=f"xn2_{g}")
            qa.dma_start(out=t, in_=xn[g * 2 * P:(g + 1) * 2 * P, :]
                         .rearrange("(c p) d -> p c d", p=P))
            xn2.append(t)
            t2 = sb.tile([P, 2, RPC], BF, name=f"aT2_{g}", tag=f"aT2_{g}")
            qb.dma_start(out=t2, in_=aT[g * 2 * P:(g + 1) * 2 * P, :]
                         .rearrange("(c p) d -> p c d", p=P))
            aT2.append(t2)
        xTs = []
        for k in range(KCH_IN):
            t = sb.tile([P, N_NODES], BF, name=f"xTs{k}", tag=f"xTs{k}")
            eng = nc.sync if k == 0 else nc.scalar
            eng.dma_start(out=t, in_=xT[k * P:(k + 1) * P, :])
            xTs.append(t)
        w2a = sb.tile([P, KCH_HID, HID], BF, name="w2a", tag="w2a")
        nc.sync.dma_start(out=w2a, in_=w2.rearrange("(c p) d -> p c d", p=P))
        lwa = sb.tile([P, KCH_HID, CLS], BF, name="lwa", tag="lwa")
        nc.scalar.dma_start(out=lwa, in_=lw.rearrange("(c p) d -> p c d", p=P))
        opwa = sb.tile([P, KCH_IN, IN_DIM], BF, name="opwa", tag="opwa")
        nc.sync.dma_start(out=opwa, in_=opw.rearrange("(c p) d -> p c d", p=P))
        pwa = sb.tile([P, KCH_IN, CLS], BF, name="pwa", tag="pwa")
        nc.scalar.dma_start(out=pwa, in_=pw.rearrange("(c p) d -> p c d", p=P))
        brows = sb.tile([1, CLS], BF, name="brows", tag="brows")
        nc.scalar.dma_start(out=brows, in_=brow[:, :])
        ones_bf = sb.tile([P, P], BF, name="ones_bf", tag="ones_bf")
        nc.vector.memset(ones_bf, 1.0)

        # bias slices (per-partition scalars)
        b1c = [bps[:, m:m + 1] for m in range(0, 4)]
        b2c = [bps[:, 4 + m:5 + m] for m in range(0, 4)]
        bqc = [bps[:, 8 + m:9 + m] for m in range(0, 2)]

        # ---------------- Phase C': axT = (A@X)^T = Xn^T-contract @ A^T ----
        # out[m=feat, n=row]; K = nodes, streamed over the 32 (xns, aTs)
        # chunk pairs so compute starts as soon as chunk 0 lands.
        axp = [pp.tile([P, RPC], F32, name=f"axp{m}", tag="mm", bufs=2)
               for m in range(KCH_IN)]
        for kc in range(NT):
            for m in range(KCH_IN):
                nc.tensor.matmul(out=axp[m],
                                 lhsT=xn2[kc // 2][:, kc % 2, m * P:(m + 1) * P],
                                 rhs=aT2[kc // 2][:, kc % 2, :],
                                 start=(kc == 0), stop=(kc == NT - 1))
        axs = []
        for m in range(KCH_IN):
            t = sb.tile([P, RPC], BF, name=f"axs{m}", tag=f"axs{m}")
            nc.vector.tensor_copy(out=t, in_=axp[m])
            axs.append(t)

        # ---------------- Phase D': h1^T = relu(W1^T-contract @ axT + b1) --
        h1Ts = []
        for m in range(KCH_HID):
            t = sb.tile([P, RPC], BF, name=f"h1T{m}", tag=f"h1T{m}")
            h1Ts.append(t)
            pt = pp.tile([P, RPC], F32, name=f"c1p{m}", tag="mm", bufs=2)
            for k in range(KCH_IN):
                nc.tensor.matmul(out=pt, lhsT=w1a[:, k, m * P:(m + 1) * P],
                                 rhs=axs[k], start=(k == 0),
                                 stop=(k == KCH_IN - 1))
            nc.scalar.activation(t, pt, AF.Relu, bias=b1c[m])

        # ---------------- Phase E: H2pre_c = h1_c @ W2 ; chunked AllGather -
        # AllGather is split into 2 feature-column chunks so conv2's first two
        # m-tiles only wait for chunk 0 — the collective pipelines vs compute.
        FCH = 2
        FW = HID // FCH  # 256 features per chunk
        ag_ins = [dr.tile([RPC, FW], BF, name=f"ag_in{f}", tag=f"ag_in{f}")
                  for f in range(FCH)]
        ag_outs = [dr.tile([N_NODES, FW], BF, name=f"ag_out{f}", tag=f"ag_out{f}")
                   for f in range(FCH)]
        for m in range(MT_Q):
            hc = sb.tile([P, HID], BF, name=f"hc{m}", tag="hc", bufs=2)
            pt = pp.tile([P, HID], F32, name=f"h2p{m}", tag="mm", bufs=2)
            for k in range(KCH_HID):
                nc.tensor.matmul(out=pt, lhsT=h1Ts[k][:, m * P:(m + 1) * P],
                                 rhs=w2a[:, k, :], start=(k == 0), stop=(k == KCH_HID - 1))
            nc.vector.tensor_copy(out=hc, in_=pt)
            for f in range(FCH):
                nc.sync.dma_start(out=ag_ins[f][m * P:(m + 1) * P, :],
                                  in_=hc[:, f * FW:(f + 1) * FW])
        # Reload: chunk f -> 16 tiles [128, 512]; tile g holds node-chunks
        # 2g, 2g+1 (256 features each) side by side. Each chunk's reloads are
        # emitted right after its collective and spread over two DMA queues so
        # chunk 0's tiles land while chunk 1 is still on the wire.
        H2s = [[None] * 16 for _ in range(FCH)]
        for f in range(FCH):
            nc.gpsimd.collective_compute(
                "AllGather", mybir.AluOpType.bypass,
                replica_groups=[list(range(NC))],
                ins=[ag_ins[f].opt()], outs=[ag_outs[f].opt()])
            for g in range(16):
                t = sb.tile([P, HID], BF, name=f"H2_{f}_{g}", tag=f"H{f * 16 + g}")
                H2s[f][g] = t
                eng = nc.sync if g % 2 == 0 else nc.gpsimd
                eng.dma_start(
                    out=t.rearrange("p (c d) -> p c d", c=2),
                    in_=ag_outs[f][g * 2 * P:(g + 1) * 2 * P, :]
                        .rearrange("(c p) d -> p c d", p=P))

        # ---------------- Phase B: qT, kT, v (runs during the AllGather) ---
        # Per-head zero-padded q tiles: qZs[tl][hh] holds (q + bq) for head
        # 2*tl+hh in partition rows 64*hh..64*hh+63 and ZEROS elsewhere, so
        # the score matmul against the packed kT tile runs as a full-mode
        # K=128 matmul (no 64-row array tiling, no PE mode-switch drains).
        qZs = []
        for m in range(2):
            row = []
            for hh in range(2):
                t = sb.tile([P, RPC], BF, name=f"qZ{m}_{hh}", tag=f"qZ{m}_{hh}")
                nc.vector.memset(t, 0.0)
                row.append(t)
            qZs.append(row)
        for m in range(2):
            pt = pp.tile([P, RPC], F32, name=f"q_ps{m}", tag="mm", bufs=2)
            for k in range(KCH_IN):
                nc.tensor.matmul(out=pt, lhsT=ipTs[k][:, m * P:(m + 1) * P],
                                 rhs=xqa[:, k, :], start=(k == 0), stop=(k == KCH_IN - 1))
            for hh in range(2):
                bpart = HDIM * hh
                nc.vector.tensor_tensor(
                    out=qZs[m][hh][bpart:bpart + HDIM, :],
                    in0=pt[bpart:bpart + HDIM, :],
                    in1=bqc[m][bpart:bpart + HDIM, :].to_broadcast([HDIM, RPC]),
                    op=mybir.AluOpType.add)
        # kT[256kd, 4096] — k-bias dropped (softmax-invariant).
        kTs = []
        for m in range(2):
            t = sb.tile([P, N_NODES], BF, name=f"kTs{m}", tag=f"kTs{m}")
            kTs.append(t)
            for n in range(NT // 4):  # 8 chunks of 512
                pt = pp.tile([P, 512], F32, name=f"k_ps{m}_{n}", tag="mm", bufs=2)
                for k in range(KCH_IN):
                    nc.tensor.matmul(out=pt,
                                     lhsT=ipTs[k][:, IN_DIM + m * P:IN_DIM + (m + 1) * P],
                                     rhs=xTs[k][:, n * 512:(n + 1) * 512],
                                     start=(k == 0), stop=(k == KCH_IN - 1))
                nc.vector.tensor_copy(out=t[:, n * 512:(n + 1) * 512], in_=pt)
        # v' tiles [128, 260]: head h cols [65h, 65h+64), col 65h+64 stays 1.0
        # (v-bias folded into brow on the host).
        vps = []
        for i in range(NT):
            t = sb.tile([P, HEADS * (HDIM + 1)], BF, name=f"vps{i}", tag=f"vps{i}")
            nc.vector.memset(t, 1.0)
            vps.append(t)
            pt = pp.tile([P, IN_DIM], F32, name=f"v_ps{i}", tag="mm", bufs=2)
            for k in range(KCH_IN):
                nc.tensor.matmul(out=pt, lhsT=xTs[k][:, i * P:(i + 1) * P],
                                 rhs=ipTs[k][:, 2 * IN_DIM:3 * IN_DIM],
                                 start=(k == 0), stop=(k == KCH_IN - 1))
            nc.vector.tensor_copy(
                out=t.rearrange("p (h d) -> p h d", h=HEADS)[:, :, 0:HDIM],
                in_=pt.rearrange("p (h d) -> p h d", h=HEADS))

        # ---------------- Phase I: Pc = opw^T @ pw (combined out projection)
        Pcs = []
        for m in range(2):
            t = sb.tile([P, CLS], BF, name=f"Pcs{m}", tag=f"Pcs{m}")
            Pcs.append(t)
            pt = pp.tile([P, CLS], F32, name=f"pc_ps{m}", tag="mm", bufs=2)
            for k in range(KCH_IN):
                nc.tensor.matmul(out=pt, lhsT=opwa[:, k, m * P:(m + 1) * P],
                                 rhs=pwa[:, k, :], start=(k == 0),
                                 stop=(k == KCH_IN - 1))
            nc.vector.tensor_copy(out=t, in_=pt)

        # ---------------- Phase J: attention (overlaps AllGather) ----------
        oTs = []
        for m in range(2):
            oTs.append(sb.tile([P, RPC], BF, name=f"oTs{m}", tag=f"oTs{m}"))
        # All 4 heads' denominator rows collected into partitions {0,32,64,96}
        # of one tile (engine writes must be 32-partition aligned) so the
        # reciprocal runs multi-lane instead of 1-lane.
        dall = sb.tile([P, RPC], F32, name="dall", tag="dall")
        nc.vector.memset(dall, 1.0)
        onums = []
        # conv2 m-tiles (emitted interleaved below): h2T[m] needs AllGather
        # chunk m//2 only. m=0/1 interleave with the tl=1 attention block
        # (their data lands mid-attention); m=2/3 follow after.
        h2Ts = []
        c2ps = []
        for m in range(KCH_HID):
            h2Ts.append(sb.tile([P, RPC], BF, name=f"h2T{m}", tag=f"h2T{m}"))
            c2ps.append(None)

        def conv2_mm(m, kc):
            if c2ps[m] is None:
                c2ps[m] = pp.tile([P, RPC], F32, name=f"c2p{m}", tag="mm",
                                  bufs=2)
            f = m // 2
            g = kc // 2
            col = (kc % 2) * 256 + (m % 2) * P
            nc.tensor.matmul(out=c2ps[m], lhsT=H2s[f][g][:, col:col + P],
                             rhs=aT2[kc // 2][:, kc % 2, :],
                             start=(kc == 0), stop=(kc == NT - 1))
            if kc == NT - 1:
                nc.scalar.activation(h2Ts[m], c2ps[m], AF.Relu, bias=b2c[m])

        for tl in range(2):
            pos = [pp.tile([HDIM + 1, RPC], F32, name=f"ob{tl}_{hh}", tag="ob",
                           bufs=2) for hh in range(2)]
            # key-chunks processed in pairs: two score matmuls land in the two
            # banks of one [128, 1024] PSUM tile, one exp covers both.
            for j in range(NT // 2):
                sss = [pp.tile([P, 2 * RPC], F32, name=f"sc{tl}_{hh}_{j}",
                               tag=f"sc{hh}", bufs=1) for hh in range(2)]
                for half in range(2):
                    i = 2 * j + half
                    for hh in range(2):
                        nc.tensor.matmul(
                            out=sss[hh][:, half * RPC:(half + 1) * RPC],
                            lhsT=kTs[tl][:, i * P:(i + 1) * P],
                            rhs=qZs[tl][hh],
                            start=True, stop=True)
                ess = []
                for hh in range(2):
                    es = sb.tile([P, 2 * RPC], BF, name=f"es{tl}_{hh}_{j}",
                                 tag=f"es{hh}", bufs=2)
                    nc.scalar.activation(es, sss[hh], AF.Exp, scale=0.125)
                    ess.append(es)
                for half in range(2):
                    i = 2 * j + half
                    for hh in range(2):
                        h = 2 * tl + hh
                        nc.tensor.matmul(
                            out=pos[hh],
                            lhsT=vps[i][:, h * (HDIM + 1):(h + 1) * (HDIM + 1)],
                            rhs=ess[hh][:, half * RPC:(half + 1) * RPC],
                            start=(i == 0), stop=(i == NT - 1))
                if tl == 1:
                    # soak tensor slack under the exp-bound cadence with
                    # conv2 m=0/1 (AllGather chunk 0 lands before tl=1).
                    for kc in range(4 * j, 4 * j + 4):
                        conv2_mm(kc // NT, kc % NT)
            for hh in range(2):
                h = 2 * tl + hh
                bpart = HDIM * hh
                po = pos[hh]
                nc.vector.tensor_copy(out=dall[32 * h:32 * h + 1, :],
                                      in_=po[HDIM:HDIM + 1, :])
                onum = sb.tile([HDIM, RPC], F32, name=f"onum{h}",
                               tag=f"onum{h}")
                nc.vector.tensor_copy(out=onum, in_=po[0:HDIM, :])
                onums.append(onum)
        # one 4-lane reciprocal for all heads, bf16 out for the K=1 broadcast
        dinv = sb.tile([P, RPC], BF, name="dinv", tag="dinv")
        with nc.allow_low_precision(reason="1/D broadcast operand in bf16"):
            nc.vector.reciprocal(dinv, dall)
        for tl in range(2):
            for hh in range(2):
                h = 2 * tl + hh
                bpart = HDIM * hh
                pb = pp.tile([HDIM, RPC], F32, name=f"pb{h}", tag="mm", bufs=2)
                nc.tensor.matmul(out=pb, lhsT=ones_bf[32 * h:32 * h + 1, 0:HDIM],
                                 rhs=dinv[32 * h:32 * h + 1, :], start=True,
                                 stop=True, tile_position=(32 * h, 0))
                nc.vector.tensor_tensor(out=oTs[tl][bpart:bpart + HDIM, :],
                                        in0=pb, in1=onums[h],
                                        op=mybir.AluOpType.mult)

        # ---------------- Phase G: conv2 m=2/3 (chunk-1 dependent) ---------
        for m in (2, 3):
            for kc in range(NT):
                conv2_mm(m, kc)

        # ---------------- Phase H/K: x_gnn, x_proj, final ------------------
        for m in range(MT_Q):
            pg = pp.tile([P, CLS], F32, name=f"xg_ps{m}", tag="mm", bufs=2)
            for k in range(KCH_HID):
                nc.tensor.matmul(out=pg, lhsT=h2Ts[k][:, m * P:(m + 1) * P],
                                 rhs=lwa[:, k, :], start=(k == 0), stop=False)
            nc.tensor.matmul(out=pg, lhsT=ones_bf[0:1, 0:P], rhs=brows,
                             start=False, stop=True)
            xg = sb.tile([P, CLS], F32, name=f"xg{m}", tag="xg", bufs=2)
            nc.vector.tensor_copy(out=xg, in_=pg)

            pj = pp.tile([P, CLS], F32, name=f"xp_ps{m}", tag="mm", bufs=2)
            for k in range(2):
                nc.tensor.matmul(out=pj, lhsT=oTs[k][:, m * P:(m + 1) * P],
                                 rhs=Pcs[k], start=(k == 0), stop=(k == 1))
            tadd = sb.tile([P, CLS], F32, name=f"tadd{m}", tag="tadd", bufs=2)
            nc.vector.tensor_tensor(out=tadd, in0=pj, in1=xg,
                                    op=mybir.AluOpType.add)
            osb = sb.tile([P, CLS], F32, name=f"osb{m}", tag="osb", bufs=2)
            nc.scalar.activation(osb, tadd, AF.Relu)
            nc.sync.dma_start(out=out[m * P:(m + 1) * P, :], in_=osb)


_CACHE = {}


def _get_compiled():
    if "nc" not in _CACHE:
        nc = bacc.Bacc("TRN2", target_bir_lowering=False, debug=False,
                       num_devices=NC)
        with tile.TileContext(nc) as tc:
            _emit(tc)
        nc.compile()
        _CACHE["nc"] = nc
    return _CACHE["nc"]


def _prepare_in_maps(inputs):
    bf16 = ml_dtypes.bfloat16
    x = np.asarray(inputs["x"], dtype=np.float32)
    ei = np.asarray(inputs["edge_index"]).astype(np.int64)

    loop = np.arange(N_NODES, dtype=np.int64)
    src = np.concatenate([ei[0], loop])
    dst = np.concatenate([ei[1], loop])
    deg = np.bincount(dst, minlength=N_NODES).astype(np.float64)
    dinv = np.where(deg > 0, 1.0 / np.sqrt(deg), 0.0)
    normv = dinv[src] * dinv[dst]
    A = np.bincount(dst * N_NODES + src, weights=normv,
                    minlength=N_NODES * N_NODES).astype(np.float32)
    A = A.reshape(N_NODES, N_NODES)

    xn = x.astype(bf16)
    xT = np.ascontiguousarray(x.T).astype(bf16)
    w1 = np.asarray(inputs["gcn1_w"], np.float32).astype(bf16)
    w2 = np.asarray(inputs["gcn2_w"], np.float32).astype(bf16)
    lwv = np.asarray(inputs["lin_w"], np.float32).astype(bf16)
    ipT = np.ascontiguousarray(np.asarray(inputs["in_proj_w"], np.float32).T).astype(bf16)
    opwv = np.asarray(inputs["out_proj_w"], np.float32).astype(bf16)
    pwv = np.asarray(inputs["proj_w"], np.float32).astype(bf16)

    b1 = np.asarray(inputs["gcn1_b"], np.float32)
    b2 = np.asarray(inputs["gcn2_b"], np.float32)
    ipb = np.asarray(inputs["in_proj_b"], np.float32)
    opw_f = np.asarray(inputs["out_proj_w"], np.float32)
    pw_f = np.asarray(inputs["proj_w"], np.float32)
    # v-bias folds through softmax (rows sum to 1) and the projections:
    bprow = (np.asarray(inputs["lin_b"], np.float32)
             + (np.asarray(inputs["out_proj_b"], np.float32)
                + ipb[2 * IN_DIM:3 * IN_DIM] @ opw_f.T) @ pw_f
             + np.asarray(inputs["proj_b"], np.float32))
    bpk = np.zeros((P, 10), np.float32)
    bpk[:, 0:4] = b1.reshape(4, P).T
    bpk[:, 4:8] = b2.reshape(4, P).T
    bpk[:, 8:10] = ipb[0:IN_DIM].reshape(2, P).T
    browv = np.ascontiguousarray(bprow[None, :]).astype(bf16)

    in_maps = []
    for c in range(NC):
        aTc = np.ascontiguousarray(A[c * RPC:(c + 1) * RPC, :].T).astype(bf16)
        in_maps.append({
            "xn": xn,
            "xT": xT,
            "xq": np.ascontiguousarray(xT[:, c * RPC:(c + 1) * RPC]),
            "aT": aTc,
            "w1": w1, "w2": w2, "lw": lwv, "ipT": ipT,
            "opw": opwv, "pw": pwv,
            "bp": bpk, "brow": browv,
        })
    return in_maps


def _run(inputs, trace=False):
    nc = _get_compiled()
    in_maps = _prepare_in_maps(inputs)
    res = run_bass_kernel_spmd(nc, in_maps, core_ids=list(range(NC)),
                               trace=trace)
    out = np.concatenate([res.results[c]["out"] for c in range(NC)], axis=0)
    return np.ascontiguousarray(out.astype(np.float32)), res


def kernel(**inputs):
    out, _ = _run(inputs, trace=False)
    return out


# revision 26
# speedup vs baseline: 1.0743x; 1.0743x over previous
"""GraphTransformer (2x GCNConv + global MHA) on 8 TRN2 NeuronCores.

Strategy
--------
Nodes (N=4096) are sharded 512/core. The GCN scatter-add is reformulated as a
dense SpMM: the host builds the normalized adjacency-with-self-loops matrix
A[dst, src] = sum(norm) once from edge_index (pure index preprocessing; every
FLOP of the network itself runs on device), and each core holds its 512-row
slice of A, transposed, as the matmul stationary operand.

Per core (rows R_c):
  conv1 is reassociated as (A @ X) @ W1 — the aggregation runs in the 256-dim
  input space (axT = X^T-nodemajor @ A_c^T, 2 m-tiles) and the W1 transform is
  applied to the 512-row slice only. This avoids both the replicated X@W1 and
  the 512-dim-space aggregation.
  H2p_c  = h1_c @ W2                 -> AllGather over cores -> H2p (full)
  h2_c^T = relu(H2p^T @ A_c^T + b2)  (conv2 aggregation)
  xg_c   = h2_c @ lin_w
  MHA: qT/kT/v from X^T (K,V replicated-computed, attention global over nodes),
       scores computed transposed sT=[keys, queries] per head, exp on ACT
       (logits are O(1) so no max-subtraction is needed), denominator obtained
       by appending a ones-column to V in the attn@V matmul. The k-bias is
       dropped (softmax-invariant) and the v-bias is folded into the output
       row bias on the host. All 4 heads' denominators are copied into one
       [4, 512] tile so the reciprocal runs 4 lanes wide, then broadcast via
       a K=1 bf16 matmul.
  out_c  = relu(xg_c + o_c @ (opw^T @ pw) + row_bias)

All matmuls run in bf16 with fp32 PSUM accumulation. The attention phase is
emitted before conv2 so it overlaps the AllGather.
"""

import os
import sys

import numpy as np
import ml_dtypes

try:
    import concourse  # noqa: F401
except ImportError:  # pragma: no cover
    sys.path.insert(0, "/opt/trn_rl_repo")

from concourse import bacc, bass, mybir, tile
from concourse.bass_utils import run_bass_kernel_spmd

P = 128
N_NODES = 4096
E_EDGES = 131072
IN_DIM = 256
HID = 512
CLS = 256
HEADS = 4
HDIM = 64
NC = 8
RPC = N_NODES // NC  # 512 rows per core

BF = mybir.dt.bfloat16
F32 = mybir.dt.float32
AF = mybir.ActivationFunctionType

KCH_IN = IN_DIM // P    # 2
KCH_HID = HID // P      # 4
NT = N_NODES // P       # 32 node tiles
MT_Q = RPC // P         # 4 query tiles per core


def _emit(tc):
    nc = tc.nc

    # ---------------- I/O ----------------
    # All multi-chunk inputs are HOST-PACKED to [P, chunks*d] so every DMA
    # descriptor reads long contiguous per-partition runs (the DMA engines
    # are row-run-rate-limited on short strided runs).
    xn = nc.dram_tensor("xn", [P, NT * IN_DIM], BF, kind="ExternalInput")
    xT = nc.dram_tensor("xT", [IN_DIM, N_NODES], BF, kind="ExternalInput")
    xq = nc.dram_tensor("xq", [P, KCH_IN * RPC], BF, kind="ExternalInput")
    aT = nc.dram_tensor("aT", [P, NT * RPC], BF, kind="ExternalInput")
    w1 = nc.dram_tensor("w1", [P, KCH_IN * HID], BF, kind="ExternalInput")
    w2 = nc.dram_tensor("w2", [P, KCH_HID * HID], BF, kind="ExternalInput")
    lw = nc.dram_tensor("lw", [P, KCH_HID * CLS], BF, kind="ExternalInput")
    ipT = nc.dram_tensor("ipT", [IN_DIM, 3 * IN_DIM], BF, kind="ExternalInput")
    opw = nc.dram_tensor("opw", [P, KCH_IN * IN_DIM], BF, kind="ExternalInput")
    pw = nc.dram_tensor("pw", [P, KCH_IN * CLS], BF, kind="ExternalInput")
    bp = nc.dram_tensor("bp", [P, 10], F32, kind="ExternalInput")
    brow = nc.dram_tensor("brow", [1, CLS], BF, kind="ExternalInput")
    out = nc.dram_tensor("out", [RPC, CLS], F32, kind="ExternalOutput")

    from contextlib import ExitStack

    with ExitStack() as ctx:
        sb = ctx.enter_context(tc.tile_pool(name="sb", bufs=1))
        pp = ctx.enter_context(tc.tile_pool(name="pp", bufs=1, space="PSUM"))
        dr = ctx.enter_context(tc.tile_pool(name="dr", bufs=1, space="DRAM"))

        # ---------------- input DMA (issued in consumer order) -------------
        # Descriptor issue costs ~0.6-1.4us on the issuing sequencer, so
        # inputs are packed into few large descriptors (2-4 row-chunks per
        # DMA, rearranged into [P, c, d] tiles) and spread over sync+scalar.
        # gpsimd is kept empty so the AllGather triggers (gpsimd-only by NRT
        # contract) fire the moment their input data is ready.
        w1a = sb.tile([P, KCH_IN, HID], BF, name="w1a", tag="w1a")
        nc.scalar.dma_start(out=w1a, in_=w1.rearrange("p (c d) -> p c d", c=KCH_IN))
        bps = sb.tile([P, 10], F32, name="bps", tag="bps")
        nc.scalar.dma_start(out=bps, in_=bp[:, :])
        xqa = sb.tile([P, KCH_IN, RPC], BF, name="xqa", tag="xqa")
        nc.scalar.dma_start(out=xqa, in_=xq.rearrange("p (c d) -> p c d", c=KCH_IN))
        ipTs = []
        for k in range(KCH_IN):
            t = sb.tile([P, 3 * IN_DIM], BF, name=f"ipTs{k}", tag=f"ipTs{k}")
            nc.scalar.dma_start(out=t, in_=ipT[k * P:(k + 1) * P, :])
            ipTs.append(t)
        # A@X stream: aT 16 descs x 256KB, xn 8 descs x 256KB, all contiguous
        # per-partition runs, alternated over the sync/scalar issue queues.
        xn4, aT2 = [], []
        for g in range(NT // 2):
            qb = nc.sync if g % 2 == 0 else nc.scalar
            t2 = sb.tile([P, 2, RPC], BF, name=f"aT2_{g}", tag=f"aT2_{g}")
            qb.dma_start(out=t2, in_=aT[:, g * 2 * RPC:(g + 1) * 2 * RPC]
                         .rearrange("p (c d) -> p c d", c=2))
            aT2.append(t2)
            if g % 2 == 0:
                gg = g // 2
                t = sb.tile([P, 4, IN_DIM], BF, name=f"xn4_{gg}", tag=f"xn4_{gg}")
                nc.scalar.dma_start(
                    out=t, in_=xn[:, gg * 4 * IN_DIM:(gg + 1) * 4 * IN_DIM]
                    .rearrange("p (c d) -> p c d", c=4))
                xn4.append(t)
        xT4 = []
        for k in range(KCH_IN):
            row = []
            for j in range(4):
                t = sb.tile([P, N_NODES // 4], BF, name=f"xT4_{k}_{j}",
                            tag=f"xT4_{k}_{j}")
                eng = nc.sync if (k + j) % 2 == 0 else nc.scalar
                eng.dma_start(out=t, in_=xT[k * P:(k + 1) * P,
                                            j * 1024:(j + 1) * 1024])
                row.append(t)
            xT4.append(row)
        w2t = []
        for h in range(2):
            t = sb.tile([P, 2, HID], BF, name=f"w2t{h}", tag=f"w2t{h}")
            eng = nc.sync if h == 0 else nc.scalar
            eng.dma_start(out=t, in_=w2[:, h * 2 * HID:(h + 1) * 2 * HID]
                          .rearrange("p (c d) -> p c d", c=2))
            w2t.append(t)
        lwa = sb.tile([P, KCH_HID, CLS], BF, name="lwa", tag="lwa")
        nc.sync.dma_start(out=lwa, in_=lw.rearrange("p (c d) -> p c d", c=KCH_HID))
        opwa = sb.tile([P, KCH_IN, IN_DIM], BF, name="opwa", tag="opwa")
        nc.sync.dma_start(out=opwa, in_=opw.rearrange("p (c d) -> p c d", c=KCH_IN))
        pwa = sb.tile([P, KCH_IN, CLS], BF, name="pwa", tag="pwa")
        nc.scalar.dma_start(out=pwa, in_=pw.rearrange("p (c d) -> p c d", c=KCH_IN))
        brows = sb.tile([1, CLS], BF, name="brows", tag="brows")
        nc.scalar.dma_start(out=brows, in_=brow[:, :])
        ones_bf = sb.tile([P, P], BF, name="ones_bf", tag="ones_bf")
        nc.vector.memset(ones_bf, 1.0)

        # bias slices (per-partition scalars)
        b1c = [bps[:, m:m + 1] for m in range(0, 4)]
        b2c = [bps[:, 4 + m:5 + m] for m in range(0, 4)]
        bqc = [bps[:, 8 + m:9 + m] for m in range(0, 2)]

        # ---------------- Phase C': axT = (A@X)^T = Xn^T-contract @ A^T ----
        # out[m=feat, n=row]; K = nodes, streamed over the 32 (xns, aTs)
        # chunk pairs so compute starts as soon as chunk 0 lands.
        axp = [pp.tile([P, RPC], F32, name=f"axp{m}", tag="mm", bufs=2)
               for m in range(KCH_IN)]
        for kc in range(NT):
            for m in range(KCH_IN):
                nc.tensor.matmul(out=axp[m],
                                 lhsT=xn4[kc // 4][:, kc % 4, m * P:(m + 1) * P],
                                 rhs=aT2[kc // 2][:, kc % 2, :],
                                 start=(kc == 0), stop=(kc == NT - 1))
        axs = []
        for m in range(KCH_IN):
            t = sb.tile([P, RPC], BF, name=f"axs{m}", tag=f"axs{m}")
            nc.vector.tensor_copy(out=t, in_=axp[m])
            axs.append(t)

        # ---------------- Phase D': h1^T = relu(W1^T-contract @ axT + b1) --
        h1Ts = []
        for m in range(KCH_HID):
            t = sb.tile([P, RPC], BF, name=f"h1T{m}", tag=f"h1T{m}")
            h1Ts.append(t)
            pt = pp.tile([P, RPC], F32, name=f"c1p{m}", tag="mm", bufs=2)
            for k in range(KCH_IN):
                nc.tensor.matmul(out=pt, lhsT=w1a[:, k, m * P:(m + 1) * P],
                                 rhs=axs[k], start=(k == 0),
                                 stop=(k == KCH_IN - 1))
            nc.scalar.activation(t, pt, AF.Relu, bias=b1c[m])

        # ---------------- Phase E: H2pre_c = h1_c @ W2 ; chunked AllGather -
        # AllGather is split into 2 feature-column chunks so conv2's first two
        # m-tiles only wait for chunk 0 — the collective pipelines vs compute.
        FCH = 2
        FW = HID // FCH  # 256 features per chunk
        ag_ins = [dr.tile([RPC, FW], BF, name=f"ag_in{f}", tag=f"ag_in{f}")
                  for f in range(FCH)]
        ag_outs = [dr.tile([N_NODES, FW], BF, name=f"ag_out{f}", tag=f"ag_out{f}")
                   for f in range(FCH)]
        for m in range(MT_Q):
            hc = sb.tile([P, HID], BF, name=f"hc{m}", tag="hc", bufs=2)
            pt = pp.tile([P, HID], F32, name=f"h2p{m}", tag="mm", bufs=2)
            for k in range(KCH_HID):
                nc.tensor.matmul(out=pt, lhsT=h1Ts[k][:, m * P:(m + 1) * P],
                                 rhs=w2t[k // 2][:, k % 2, :], start=(k == 0),
                                 stop=(k == KCH_HID - 1))
            nc.vector.tensor_copy(out=hc, in_=pt)
            for f in range(FCH):
                nc.sync.dma_start(out=ag_ins[f][m * P:(m + 1) * P, :],
                                  in_=hc[:, f * FW:(f + 1) * FW])
        # Reload: chunk f -> 16 tiles [128, 512]; tile g holds node-chunks
        # 2g, 2g+1 (256 features each) side by side. Each chunk's reloads are
        # emitted right after its collective and spread over two DMA queues so
        # chunk 0's tiles land while chunk 1 is still on the wire.
        H2s = [[None] * 16 for _ in range(FCH)]
        for f in range(FCH):
            nc.gpsimd.collective_compute(
                "AllGather", mybir.AluOpType.bypass,
                replica_groups=[list(range(NC))],
                ins=[ag_ins[f].opt()], outs=[ag_outs[f].opt()])
            for g in range(16):
                t = sb.tile([P, HID], BF, name=f"H2_{f}_{g}", tag=f"H{f * 16 + g}")
                H2s[f][g] = t
                eng = nc.sync if g % 2 == 0 else nc.gpsimd
                eng.dma_start(
                    out=t.rearrange("p (c d) -> p c d", c=2),
                    in_=ag_outs[f][g * 2 * P:(g + 1) * 2 * P, :]
                        .rearrange("(c p) d -> p c d", p=P))

        # ---------------- Phase B: qT, kT, v (runs during the AllGather) ---
        # Per-head zero-padded q tiles: qZs[tl][hh] holds (q + bq) for head
        # 2*tl+hh in partition rows 64*hh..64*hh+63 and ZEROS elsewhere, so
        # the score matmul against the packed kT tile runs as a full-mode
        # K=128 matmul (no 64-row array tiling, no PE mode-switch drains).
        qZs = []
        for m in range(2):
            row = []
            for hh in range(2):
                t = sb.tile([P, RPC], BF, name=f"qZ{m}_{hh}", tag=f"qZ{m}_{hh}")
                nc.vector.memset(t, 0.0)
                row.append(t)
            qZs.append(row)
        for m in range(2):
            pt = pp.tile([P, RPC], F32, name=f"q_ps{m}", tag="mm", bufs=2)
            for k in range(KCH_IN):
                nc.tensor.matmul(out=pt, lhsT=ipTs[k][:, m * P:(m + 1) * P],
                                 rhs=xqa[:, k, :], start=(k == 0), stop=(k == KCH_IN - 1))
            for hh in range(2):
                bpart = HDIM * hh
                nc.vector.tensor_tensor(
                    out=qZs[m][hh][bpart:bpart + HDIM, :],
                    in0=pt[bpart:bpart + HDIM, :],
                    in1=bqc[m][bpart:bpart + HDIM, :].to_broadcast([HDIM, RPC]),
                    op=mybir.AluOpType.add)
        # kT[256kd, 4096] — k-bias dropped (softmax-invariant).
        kTs = []
        for m in range(2):
            t = sb.tile([P, N_NODES], BF, name=f"kTs{m}", tag=f"kTs{m}")
            kTs.append(t)
            for n in range(NT // 4):  # 8 chunks of 512
                pt = pp.tile([P, 512], F32, name=f"k_ps{m}_{n}", tag="mm", bufs=2)
                for k in range(KCH_IN):
                    nc.tensor.matmul(out=pt,
                                     lhsT=ipTs[k][:, IN_DIM + m * P:IN_DIM + (m + 1) * P],
                                     rhs=xT4[k][n // 2][:, (n % 2) * 512:
                                                        (n % 2) * 512 + 512],
                                     start=(k == 0), stop=(k == KCH_IN - 1))
                nc.vector.tensor_copy(out=t[:, n * 512:(n + 1) * 512], in_=pt)
        # v' tiles [128, 260]: head h cols [65h, 65h+64), col 65h+64 stays 1.0
        # (v-bias folded into brow on the host).
        vps = []
        for i in range(NT):
            t = sb.tile([P, HEADS * (HDIM + 1)], BF, name=f"vps{i}", tag=f"vps{i}")
            nc.vector.memset(t, 1.0)
            vps.append(t)
            pt = pp.tile([P, IN_DIM], F32, name=f"v_ps{i}", tag="mm", bufs=2)
            for k in range(KCH_IN):
                nc.tensor.matmul(
                    out=pt,
                    lhsT=xT4[k][i // 8][:, (i % 8) * P:(i % 8) * P + P],
                    rhs=ipTs[k][:, 2 * IN_DIM:3 * IN_DIM],
                    start=(k == 0), stop=(k == KCH_IN - 1))
            nc.vector.tensor_copy(
                out=t.rearrange("p (h d) -> p h d", h=HEADS)[:, :, 0:HDIM],
                in_=pt.rearrange("p (h d) -> p h d", h=HEADS))

        # ---------------- Phase I: Pc = opw^T @ pw (combined out projection)
        Pcs = []
        for m in range(2):
            t = sb.tile([P, CLS], BF, name=f"Pcs{m}", tag=f"Pcs{m}")
            Pcs.append(t)
            pt = pp.tile([P, CLS], F32, name=f"pc_ps{m}", tag="mm", bufs=2)
            for k in range(KCH_IN):
                nc.tensor.matmul(out=pt, lhsT=opwa[:, k, m * P:(m + 1) * P],
                                 rhs=pwa[:, k, :], start=(k == 0),
                                 stop=(k == KCH_IN - 1))
            nc.vector.tensor_copy(out=t, in_=pt)

        # ---------------- Phase J: attention (overlaps AllGather) ----------
        oTs = []
        for m in range(2):
            oTs.append(sb.tile([P, RPC], BF, name=f"oTs{m}", tag=f"oTs{m}"))
        # All 4 heads' denominator rows collected into partitions {0,32,64,96}
        # of one tile (engine writes must be 32-partition aligned) so the
        # reciprocal runs multi-lane instead of 1-lane.
        dall = sb.tile([P, RPC], F32, name="dall", tag="dall")
        nc.vector.memset(dall, 1.0)
        onums = []
        # conv2 m-tiles (emitted interleaved below): h2T[m] needs AllGather
        # chunk m//2 only. m=0/1 interleave with the tl=1 attention block
        # (their data lands mid-attention); m=2/3 follow after.
        h2Ts = []
        c2ps = []
        for m in range(KCH_HID):
            h2Ts.append(sb.tile([P, RPC], BF, name=f"h2T{m}", tag=f"h2T{m}"))
            c2ps.append(None)

        def conv2_mm(m, kc):
            if c2ps[m] is None:
                c2ps[m] = pp.tile([P, RPC], F32, name=f"c2p{m}", tag="mm",
                                  bufs=2)
            f = m // 2
            g = kc // 2
            col = (kc % 2) * 256 + (m % 2) * P
            nc.tensor.matmul(out=c2ps[m], lhsT=H2s[f][g][:, col:col + P],
                             rhs=aT2[kc // 2][:, kc % 2, :],
                             start=(kc == 0), stop=(kc == NT - 1))
            if kc == NT - 1:
                nc.scalar.activation(h2Ts[m], c2ps[m], AF.Relu, bias=b2c[m])

        for tl in range(2):
            pos = [pp.tile([HDIM + 1, RPC], F32, name=f"ob{tl}_{hh}", tag="ob",
                           bufs=2) for hh in range(2)]
            # key-chunks processed in pairs: two score matmuls land in the two
            # banks of one [128, 1024] PSUM tile, one exp covers both.
            for j in range(NT // 2):
                sss = [pp.tile([P, 2 * RPC], F32, name=f"sc{tl}_{hh}_{j}",
                               tag=f"sc{hh}", bufs=1) for hh in range(2)]
                for half in range(2):
                    i = 2 * j + half
                    for hh in range(2):
                        nc.tensor.matmul(
                            out=sss[hh][:, half * RPC:(half + 1) * RPC],
                            lhsT=kTs[tl][:, i * P:(i + 1) * P],
                            rhs=qZs[tl][hh],
                            start=True, stop=True)
                ess = []
                for hh in range(2):
                    es = sb.tile([P, 2 * RPC], BF, name=f"es{tl}_{hh}_{j}",
                                 tag=f"es{hh}", bufs=2)
                    nc.scalar.activation(es, sss[hh], AF.Exp, scale=0.125)
                    ess.append(es)
                for half in range(2):
                    i = 2 * j + half
                    for hh in range(2):
                        h = 2 * tl + hh
                        nc.tensor.matmul(
                            out=pos[hh],
                            lhsT=vps[i][:, h * (HDIM + 1):(h + 1) * (HDIM + 1)],
                            rhs=ess[hh][:, half * RPC:(half + 1) * RPC],
                            start=(i == 0), stop=(i == NT - 1))
            for hh in range(2):
                h = 2 * tl + hh
                bpart = HDIM * hh
                po = pos[hh]
                nc.vector.tensor_copy(out=dall[32 * h:32 * h + 1, :],
                                      in_=po[HDIM:HDIM + 1, :])
                onum = sb.tile([HDIM, RPC], F32, name=f"onum{h}",
                               tag=f"onum{h}")
                nc.vector.tensor_copy(out=onum, in_=po[0:HDIM, :])
                onums.append(onum)
        # one 4-lane reciprocal for all heads, bf16 out for the K=1 broadcast
        dinv = sb.tile([P, RPC], BF, name="dinv", tag="dinv")
        with nc.allow_low_precision(reason="1/D broadcast operand in bf16"):
            nc.vector.reciprocal(dinv, dall)
        for tl in range(2):
            for hh in range(2):
                h = 2 * tl + hh
                bpart = HDIM * hh
                pb = pp.tile([HDIM, RPC], F32, name=f"pb{h}", tag="mm", bufs=2)
                nc.tensor.matmul(out=pb, lhsT=ones_bf[32 * h:32 * h + 1, 0:HDIM],
                                 rhs=dinv[32 * h:32 * h + 1, :], start=True,
                                 stop=True, tile_position=(32 * h, 0))
                nc.vector.tensor_tensor(out=oTs[tl][bpart:bpart + HDIM, :],
                                        in0=pb, in1=onums[h],
                                        op=mybir.AluOpType.mult)

        # ---------------- Phase G: conv2 (after attention) -----------------
        for m in range(KCH_HID):
            for kc in range(NT):
                conv2_mm(m, kc)

        # ---------------- Phase H/K: x_gnn, x_proj, final ------------------
        for m in range(MT_Q):
            pg = pp.tile([P, CLS], F32, name=f"xg_ps{m}", tag="mm", bufs=2)
            for k in range(KCH_HID):
                nc.tensor.matmul(out=pg, lhsT=h2Ts[k][:, m * P:(m + 1) * P],
                                 rhs=lwa[:, k, :], start=(k == 0), stop=False)
            nc.tensor.matmul(out=pg, lhsT=ones_bf[0:1, 0:P], rhs=brows,
                             start=False, stop=True)
            xg = sb.tile([P, CLS], F32, name=f"xg{m}", tag="xg", bufs=2)
            nc.vector.tensor_copy(out=xg, in_=pg)

            pj = pp.tile([P, CLS], F32, name=f"xp_ps{m}", tag="mm", bufs=2)
            for k in range(2):
                nc.tensor.matmul(out=pj, lhsT=oTs[k][:, m * P:(m + 1) * P],
                                 rhs=Pcs[k], start=(k == 0), stop=(k == 1))
            tadd = sb.tile([P, CLS], F32, name=f"tadd{m}", tag="tadd", bufs=2)
            nc.vector.tensor_tensor(out=tadd, in0=pj, in1=xg,
                                    op=mybir.AluOpType.add)
            osb = sb.tile([P, CLS], F32, name=f"osb{m}", tag="osb", bufs=2)
            nc.scalar.activation(osb, tadd, AF.Relu)
            nc.sync.dma_start(out=out[m * P:(m + 1) * P, :], in_=osb)


_CACHE = {}


def _get_compiled():
    if "nc" not in _CACHE:
        nc = bacc.Bacc("TRN2", target_bir_lowering=False, debug=False,
                       num_devices=NC)
        with tile.TileContext(nc) as tc:
            _emit(tc)
        nc.compile()
        _CACHE["nc"] = nc
    return _CACHE["nc"]


def _prepare_in_maps(inputs):
    bf16 = ml_dtypes.bfloat16
    x = np.asarray(inputs["x"], dtype=np.float32)
    ei = np.asarray(inputs["edge_index"]).astype(np.int64)

    loop = np.arange(N_NODES, dtype=np.int64)
    src = np.concatenate([ei[0], loop])
    dst = np.concatenate([ei[1], loop])
    deg = np.bincount(dst, minlength=N_NODES).astype(np.float64)
    dinv = np.where(deg > 0, 1.0 / np.sqrt(deg), 0.0)
    normv = dinv[src] * dinv[dst]
    A = np.bincount(dst * N_NODES + src, weights=normv,
                    minlength=N_NODES * N_NODES).astype(np.float32)
    A = A.reshape(N_NODES, N_NODES)

    def pack(arr):
        # [C*P, D] -> [P, C*D]: partition-row p holds chunks c contiguously,
        # so each DMA descriptor reads long contiguous per-partition runs.
        cp, d = arr.shape
        c = cp // P
        return np.ascontiguousarray(
            arr.reshape(c, P, d).transpose(1, 0, 2).reshape(P, c * d))

    xn = pack(x.astype(bf16))
    xT = np.ascontiguousarray(x.T).astype(bf16)
    w1 = pack(np.asarray(inputs["gcn1_w"], np.float32).astype(bf16))
    w2 = pack(np.asarray(inputs["gcn2_w"], np.float32).astype(bf16))
    lwv = pack(np.asarray(inputs["lin_w"], np.float32).astype(bf16))
    ipT = np.ascontiguousarray(np.asarray(inputs["in_proj_w"], np.float32).T).astype(bf16)
    opwv = pack(np.asarray(inputs["out_proj_w"], np.float32).astype(bf16))
    pwv = pack(np.asarray(inputs["proj_w"], np.float32).astype(bf16))

    b1 = np.asarray(inputs["gcn1_b"], np.float32)
    b2 = np.asarray(inputs["gcn2_b"], np.float32)
    ipb = np.asarray(inputs["in_proj_b"], np.float32)
    opw_f = np.asarray(inputs["out_proj_w"], np.float32)
    pw_f = np.asarray(inputs["proj_w"], np.float32)
    # v-bias folds through softmax (rows sum to 1) and the projections:
    bprow = (np.asarray(inputs["lin_b"], np.float32)
             + (np.asarray(inputs["out_proj_b"], np.float32)
                + ipb[2 * IN_DIM:3 * IN_DIM] @ opw_f.T) @ pw_f
             + np.asarray(inputs["proj_b"], np.float32))
    bpk = np.zeros((P, 10), np.float32)
    bpk[:, 0:4] = b1.reshape(4, P).T
    bpk[:, 4:8] = b2.reshape(4, P).T
    bpk[:, 8:10] = ipb[0:IN_DIM].reshape(2, P).T
    browv = np.ascontiguousarray(bprow[None, :]).astype(bf16)

    in_maps = []
    for c in range(NC):
        aTc = pack(np.ascontiguousarray(A[c * RPC:(c + 1) * RPC, :].T)
                   .astype(bf16))
        in_maps.append({
            "xn": xn,
            "xT": xT,
            "xq": pack(np.ascontiguousarray(xT[:, c * RPC:(c + 1) * RPC])),
            "aT": aTc,
            "w1": w1, "w2": w2, "lw": lwv, "ipT": ipT,
            "opw": opwv, "pw": pwv,
            "bp": bpk, "brow": browv,
        })
    return in_maps


def _run(inputs, trace=False):
    nc = _get_compiled()
    in_maps = _prepare_in_maps(inputs)
    res = run_bass_kernel_spmd(nc, in_maps, core_ids=list(range(NC)),
                               trace=trace)
    out = np.concatenate([res.results[c]["out"] for c in range(NC)], axis=0)
    return np.ascontiguousarray(out.astype(np.float32)), res


def kernel(**inputs):
    out, _ = _run(inputs, trace=False)
    return out


# revision 27
# speedup vs baseline: 1.1391x; 1.0603x over previous
"""GraphTransformer (2x GCNConv + global MHA) on 8 TRN2 NeuronCores.

Strategy
--------
Nodes (N=4096) are sharded 512/core. The GCN scatter-add is reformulated as a
dense SpMM: the host builds the normalized adjacency-with-self-loops matrix
A[dst, src] = sum(norm) once from edge_index (pure index preprocessing; every
FLOP of the network itself runs on device), and each core holds its 512-row
slice of A, transposed, as the matmul stationary operand.

Per core (rows R_c):
  conv1 is reassociated as (A @ X) @ W1 — the aggregation runs in the 256-dim
  input space (axT = X^T-nodemajor @ A_c^T, 2 m-tiles) and the W1 transform is
  applied to the 512-row slice only. This avoids both the replicated X@W1 and
  the 512-dim-space aggregation.
  H2p_c  = h1_c @ W2                 -> AllGather over cores -> H2p (full)
  h2_c^T = relu(H2p^T @ A_c^T + b2)  (conv2 aggregation)
  xg_c   = h2_c @ lin_w
  MHA: qT/kT/v from X^T (K,V replicated-computed, attention global over nodes),
       scores computed transposed sT=[keys, queries] per head, exp on ACT
       (logits are O(1) so no max-subtraction is needed), denominator obtained
       by appending a ones-column to V in the attn@V matmul. The k-bias is
       dropped (softmax-invariant) and the v-bias is folded into the output
       row bias on the host. All 4 heads' denominators are copied into one
       [4, 512] tile so the reciprocal runs 4 lanes wide, then broadcast via
       a K=1 bf16 matmul.
  out_c  = relu(xg_c + o_c @ (opw^T @ pw) + row_bias)

All matmuls run in bf16 with fp32 PSUM accumulation. The attention phase is
emitted before conv2 so it overlaps the AllGather.
"""

import os
import sys

import numpy as np
import ml_dtypes

try:
    import concourse  # noqa: F401
except ImportError:  # pragma: no cover
    sys.path.insert(0, "/opt/trn_rl_repo")

from concourse import bacc, bass, mybir, tile
from concourse.bass_utils import run_bass_kernel_spmd

P = 128
N_NODES = 4096
E_EDGES = 131072
IN_DIM = 256
HID = 512
CLS = 256
HEADS = 4
HDIM = 64
NC = 8
RPC = N_NODES // NC  # 512 rows per core

BF = mybir.dt.bfloat16
F32 = mybir.dt.float32
AF = mybir.ActivationFunctionType

KCH_IN = IN_DIM // P    # 2
KCH_HID = HID // P      # 4
NT = N_NODES // P       # 32 node tiles
MT_Q = RPC // P         # 4 query tiles per core


def _emit(tc):
    nc = tc.nc

    # ---------------- I/O ----------------
    # All multi-chunk inputs are HOST-PACKED to [P, chunks*d] so every DMA
    # descriptor reads long contiguous per-partition runs (the DMA engines
    # are row-run-rate-limited on short strided runs).
    xn = nc.dram_tensor("xn", [P, NT * IN_DIM], BF, kind="ExternalInput")
    xT = nc.dram_tensor("xT", [IN_DIM, N_NODES], BF, kind="ExternalInput")
    xq = nc.dram_tensor("xq", [P, KCH_IN * RPC], BF, kind="ExternalInput")
    aT = nc.dram_tensor("aT", [P, NT * RPC], BF, kind="ExternalInput")
    w1 = nc.dram_tensor("w1", [P, KCH_IN * HID], BF, kind="ExternalInput")
    w2 = nc.dram_tensor("w2", [P, KCH_HID * HID], BF, kind="ExternalInput")
    lw = nc.dram_tensor("lw", [P, KCH_HID * CLS], BF, kind="ExternalInput")
    ipT = nc.dram_tensor("ipT", [IN_DIM, 3 * IN_DIM], BF, kind="ExternalInput")
    opw = nc.dram_tensor("opw", [P, KCH_IN * IN_DIM], BF, kind="ExternalInput")
    pw = nc.dram_tensor("pw", [P, KCH_IN * CLS], BF, kind="ExternalInput")
    bp = nc.dram_tensor("bp", [P, 10], F32, kind="ExternalInput")
    brow = nc.dram_tensor("brow", [1, CLS], BF, kind="ExternalInput")
    out = nc.dram_tensor("out", [RPC, CLS], F32, kind="ExternalOutput")

    from contextlib import ExitStack

    with ExitStack() as ctx:
        sb = ctx.enter_context(tc.tile_pool(name="sb", bufs=1))
        pp = ctx.enter_context(tc.tile_pool(name="pp", bufs=1, space="PSUM"))
        dr = ctx.enter_context(tc.tile_pool(name="dr", bufs=1, space="DRAM"))

        # ---------------- input DMA (issued in consumer order) -------------
        # Descriptor issue costs ~0.6-1.4us on the issuing sequencer, so
        # inputs are packed into few large descriptors (2-4 row-chunks per
        # DMA, rearranged into [P, c, d] tiles) and spread over sync+scalar.
        # gpsimd is kept empty so the AllGather triggers (gpsimd-only by NRT
        # contract) fire the moment their input data is ready.
        # A@X stream first, in exact consumption order (per 4 kc-chunks:
        # two aT2 descs + one xn4 desc), round-robin over sync/scalar.
        qs = [nc.sync, nc.scalar]
        xn4 = [sb.tile([P, 4, IN_DIM], BF, name=f"xn4_{gg}", tag=f"xn4_{gg}")
               for gg in range(NT // 4)]
        aT2 = [sb.tile([P, 2, RPC], BF, name=f"aT2_{g}", tag=f"aT2_{g}")
               for g in range(NT // 2)]
        qi = 0

        def issue(out_t, in_ap):
            nonlocal qi
            qs[qi % 2].dma_start(out=out_t, in_=in_ap)
            qi += 1

        for gg in range(NT // 4):
            issue(aT2[2 * gg], aT[:, (2 * gg) * 2 * RPC:(2 * gg + 1) * 2 * RPC]
                  .rearrange("p (c d) -> p c d", c=2))
            issue(xn4[gg], xn[:, gg * 4 * IN_DIM:(gg + 1) * 4 * IN_DIM]
                  .rearrange("p (c d) -> p c d", c=4))
            issue(aT2[2 * gg + 1],
                  aT[:, (2 * gg + 1) * 2 * RPC:(2 * gg + 2) * 2 * RPC]
                  .rearrange("p (c d) -> p c d", c=2))
        w1a = sb.tile([P, KCH_IN, HID], BF, name="w1a", tag="w1a")
        issue(w1a, w1.rearrange("p (c d) -> p c d", c=KCH_IN))
        bps = sb.tile([P, 10], F32, name="bps", tag="bps")
        issue(bps, bp[:, :])
        xqa = sb.tile([P, KCH_IN, RPC], BF, name="xqa", tag="xqa")
        issue(xqa, xq.rearrange("p (c d) -> p c d", c=KCH_IN))
        ipTs = []
        for k in range(KCH_IN):
            t = sb.tile([P, 3 * IN_DIM], BF, name=f"ipTs{k}", tag=f"ipTs{k}")
            issue(t, ipT[k * P:(k + 1) * P, :])
            ipTs.append(t)
        xT4 = []
        for k in range(KCH_IN):
            row = []
            for j in range(4):
                t = sb.tile([P, N_NODES // 4], BF, name=f"xT4_{k}_{j}",
                            tag=f"xT4_{k}_{j}")
                issue(t, xT[k * P:(k + 1) * P, j * 1024:(j + 1) * 1024])
                row.append(t)
            xT4.append(row)
        w2t = []
        for h in range(2):
            t = sb.tile([P, 2, HID], BF, name=f"w2t{h}", tag=f"w2t{h}")
            issue(t, w2[:, h * 2 * HID:(h + 1) * 2 * HID]
                  .rearrange("p (c d) -> p c d", c=2))
            w2t.append(t)
        lwa = sb.tile([P, KCH_HID, CLS], BF, name="lwa", tag="lwa")
        issue(lwa, lw.rearrange("p (c d) -> p c d", c=KCH_HID))
        opwa = sb.tile([P, KCH_IN, IN_DIM], BF, name="opwa", tag="opwa")
        issue(opwa, opw.rearrange("p (c d) -> p c d", c=KCH_IN))
        pwa = sb.tile([P, KCH_IN, CLS], BF, name="pwa", tag="pwa")
        issue(pwa, pw.rearrange("p (c d) -> p c d", c=KCH_IN))
        brows = sb.tile([1, CLS], BF, name="brows", tag="brows")
        issue(brows, brow[:, :])
        ones_bf = sb.tile([P, P], BF, name="ones_bf", tag="ones_bf")
        nc.vector.memset(ones_bf, 1.0)

        # bias slices (per-partition scalars)
        b1c = [bps[:, m:m + 1] for m in range(0, 4)]
        b2c = [bps[:, 4 + m:5 + m] for m in range(0, 4)]
        bqc = [bps[:, 8 + m:9 + m] for m in range(0, 2)]

        # ---------------- Phase C': axT = (A@X)^T = Xn^T-contract @ A^T ----
        # out[m=feat, n=row]; K = nodes, streamed over the 32 (xns, aTs)
        # chunk pairs so compute starts as soon as chunk 0 lands.
        axp = [pp.tile([P, RPC], F32, name=f"axp{m}", tag="mm", bufs=2)
               for m in range(KCH_IN)]
        for kc in range(NT):
            for m in range(KCH_IN):
                nc.tensor.matmul(out=axp[m],
                                 lhsT=xn4[kc // 4][:, kc % 4, m * P:(m + 1) * P],
                                 rhs=aT2[kc // 2][:, kc % 2, :],
                                 start=(kc == 0), stop=(kc == NT - 1))
        axs = []
        for m in range(KCH_IN):
            t = sb.tile([P, RPC], BF, name=f"axs{m}", tag=f"axs{m}")
            nc.vector.tensor_copy(out=t, in_=axp[m])
            axs.append(t)

        # ---------------- Phase D': h1^T = relu(W1^T-contract @ axT + b1) --
        h1Ts = []
        for m in range(KCH_HID):
            t = sb.tile([P, RPC], BF, name=f"h1T{m}", tag=f"h1T{m}")
            h1Ts.append(t)
            pt = pp.tile([P, RPC], F32, name=f"c1p{m}", tag="mm", bufs=2)
            for k in range(KCH_IN):
                nc.tensor.matmul(out=pt, lhsT=w1a[:, k, m * P:(m + 1) * P],
                                 rhs=axs[k], start=(k == 0),
                                 stop=(k == KCH_IN - 1))
            nc.scalar.activation(t, pt, AF.Relu, bias=b1c[m])

        # ---------------- Phase E: H2pre_c = h1_c @ W2 ; chunked AllGather -
        # AllGather is split into 2 feature-column chunks so conv2's first two
        # m-tiles only wait for chunk 0 — the collective pipelines vs compute.
        FCH = 2
        FW = HID // FCH  # 256 features per chunk
        ag_ins = [dr.tile([RPC, FW], BF, name=f"ag_in{f}", tag=f"ag_in{f}")
                  for f in range(FCH)]
        ag_outs = [dr.tile([N_NODES, FW], BF, name=f"ag_out{f}", tag=f"ag_out{f}")
                   for f in range(FCH)]
        for m in range(MT_Q):
            hc = sb.tile([P, HID], BF, name=f"hc{m}", tag="hc", bufs=2)
            pt = pp.tile([P, HID], F32, name=f"h2p{m}", tag="mm", bufs=2)
            for k in range(KCH_HID):
                nc.tensor.matmul(out=pt, lhsT=h1Ts[k][:, m * P:(m + 1) * P],
                                 rhs=w2t[k // 2][:, k % 2, :], start=(k == 0),
                                 stop=(k == KCH_HID - 1))
            nc.vector.tensor_copy(out=hc, in_=pt)
            for f in range(FCH):
                nc.sync.dma_start(out=ag_ins[f][m * P:(m + 1) * P, :],
                                  in_=hc[:, f * FW:(f + 1) * FW])
        # Reload: chunk f -> 16 tiles [128, 512]; tile g holds node-chunks
        # 2g, 2g+1 (256 features each) side by side. Each chunk's reloads are
        # emitted right after its collective and spread over two DMA queues so
        # chunk 0's tiles land while chunk 1 is still on the wire.
        H2s = [[None] * 16 for _ in range(FCH)]
        for f in range(FCH):
            nc.gpsimd.collective_compute(
                "AllGather", mybir.AluOpType.bypass,
                replica_groups=[list(range(NC))],
                ins=[ag_ins[f].opt()], outs=[ag_outs[f].opt()])
            for g in range(16):
                t = sb.tile([P, HID], BF, name=f"H2_{f}_{g}", tag=f"H{f * 16 + g}")
                H2s[f][g] = t
                eng = nc.sync if g % 2 == 0 else nc.gpsimd
                eng.dma_start(
                    out=t.rearrange("p (c d) -> p c d", c=2),
                    in_=ag_outs[f][g * 2 * P:(g + 1) * 2 * P, :]
                        .rearrange("(c p) d -> p c d", p=P))

        # ---------------- Phase B: qT, kT, v (runs during the AllGather) ---
        # Per-head zero-padded q tiles: qZs[tl][hh] holds (q + bq) for head
        # 2*tl+hh in partition rows 64*hh..64*hh+63 and ZEROS elsewhere, so
        # the score matmul against the packed kT tile runs as a full-mode
        # K=128 matmul (no 64-row array tiling, no PE mode-switch drains).
        qZs = []
        for m in range(2):
            row = []
            for hh in range(2):
                t = sb.tile([P, RPC], BF, name=f"qZ{m}_{hh}", tag=f"qZ{m}_{hh}")
                nc.vector.memset(t, 0.0)
                row.append(t)
            qZs.append(row)
        for m in range(2):
            pt = pp.tile([P, RPC], F32, name=f"q_ps{m}", tag="mm", bufs=2)
            for k in range(KCH_IN):
                nc.tensor.matmul(out=pt, lhsT=ipTs[k][:, m * P:(m + 1) * P],
                                 rhs=xqa[:, k, :], start=(k == 0), stop=(k == KCH_IN - 1))
            for hh in range(2):
                bpart = HDIM * hh
                nc.vector.tensor_tensor(
                    out=qZs[m][hh][bpart:bpart + HDIM, :],
                    in0=pt[bpart:bpart + HDIM, :],
                    in1=bqc[m][bpart:bpart + HDIM, :].to_broadcast([HDIM, RPC]),
                    op=mybir.AluOpType.add)
        # kT[256kd, 4096] — k-bias dropped (softmax-invariant).
        kTs = []
        for m in range(2):
            t = sb.tile([P, N_NODES], BF, name=f"kTs{m}", tag=f"kTs{m}")
            kTs.append(t)
            for n in range(NT // 4):  # 8 chunks of 512
                pt = pp.tile([P, 512], F32, name=f"k_ps{m}_{n}", tag="mm", bufs=2)
                for k in range(KCH_IN):
                    nc.tensor.matmul(out=pt,
                                     lhsT=ipTs[k][:, IN_DIM + m * P:IN_DIM + (m + 1) * P],
                                     rhs=xT4[k][n // 2][:, (n % 2) * 512:
                                                        (n % 2) * 512 + 512],
                                     start=(k == 0), stop=(k == KCH_IN - 1))
                nc.vector.tensor_copy(out=t[:, n * 512:(n + 1) * 512], in_=pt)
        # v' tiles [128, 260]: head h cols [65h, 65h+64), col 65h+64 stays 1.0
        # (v-bias folded into brow on the host).
        vps = []
        for i in range(NT):
            t = sb.tile([P, HEADS * (HDIM + 1)], BF, name=f"vps{i}", tag=f"vps{i}")
            nc.vector.memset(t, 1.0)
            vps.append(t)
            pt = pp.tile([P, IN_DIM], F32, name=f"v_ps{i}", tag="mm", bufs=2)
            for k in range(KCH_IN):
                nc.tensor.matmul(
                    out=pt,
                    lhsT=xT4[k][i // 8][:, (i % 8) * P:(i % 8) * P + P],
                    rhs=ipTs[k][:, 2 * IN_DIM:3 * IN_DIM],
                    start=(k == 0), stop=(k == KCH_IN - 1))
            nc.vector.tensor_copy(
                out=t.rearrange("p (h d) -> p h d", h=HEADS)[:, :, 0:HDIM],
                in_=pt.rearrange("p (h d) -> p h d", h=HEADS))

        # ---------------- Phase I: Pc = opw^T @ pw (combined out projection)
        Pcs = []
        for m in range(2):
            t = sb.tile([P, CLS], BF, name=f"Pcs{m}", tag=f"Pcs{m}")
            Pcs.append(t)
            pt = pp.tile([P, CLS], F32, name=f"pc_ps{m}", tag="mm", bufs=2)
            for k in range(KCH_IN):
                nc.tensor.matmul(out=pt, lhsT=opwa[:, k, m * P:(m + 1) * P],
                                 rhs=pwa[:, k, :], start=(k == 0),
                                 stop=(k == KCH_IN - 1))
            nc.vector.tensor_copy(out=t, in_=pt)

        # ---------------- Phase J: attention (overlaps AllGather) ----------
        oTs = []
        for m in range(2):
            oTs.append(sb.tile([P, RPC], BF, name=f"oTs{m}", tag=f"oTs{m}"))
        # All 4 heads' denominator rows collected into partitions {0,32,64,96}
        # of one tile (engine writes must be 32-partition aligned) so the
        # reciprocal runs multi-lane instead of 1-lane.
        dall = sb.tile([P, RPC], F32, name="dall", tag="dall")
        nc.vector.memset(dall, 1.0)
        onums = []
        # conv2 m-tiles (emitted interleaved below): h2T[m] needs AllGather
        # chunk m//2 only. m=0/1 interleave with the tl=1 attention block
        # (their data lands mid-attention); m=2/3 follow after.
        h2Ts = []
        c2ps = []
        for m in range(KCH_HID):
            h2Ts.append(sb.tile([P, RPC], BF, name=f"h2T{m}", tag=f"h2T{m}"))
            c2ps.append(None)

        def conv2_mm(m, kc):
            if c2ps[m] is None:
                c2ps[m] = pp.tile([P, RPC], F32, name=f"c2p{m}", tag="mm",
                                  bufs=2)
            f = m // 2
            g = kc // 2
            col = (kc % 2) * 256 + (m % 2) * P
            nc.tensor.matmul(out=c2ps[m], lhsT=H2s[f][g][:, col:col + P],
                             rhs=aT2[kc // 2][:, kc % 2, :],
                             start=(kc == 0), stop=(kc == NT - 1))
            if kc == NT - 1:
                nc.scalar.activation(h2Ts[m], c2ps[m], AF.Relu, bias=b2c[m])

        for tl in range(2):
            pos = [pp.tile([HDIM + 1, RPC], F32, name=f"ob{tl}_{hh}", tag="ob",
                           bufs=2) for hh in range(2)]
            # key-chunks processed in pairs: two score matmuls land in the two
            # banks of one [128, 1024] PSUM tile, one exp covers both.
            for j in range(NT // 2):
                sss = [pp.tile([P, 2 * RPC], F32, name=f"sc{tl}_{hh}_{j}",
                               tag=f"sc{hh}", bufs=1) for hh in range(2)]
                for half in range(2):
                    i = 2 * j + half
                    for hh in range(2):
                        nc.tensor.matmul(
                            out=sss[hh][:, half * RPC:(half + 1) * RPC],
                            lhsT=kTs[tl][:, i * P:(i + 1) * P],
                            rhs=qZs[tl][hh],
                            start=True, stop=True)
                ess = []
                for hh in range(2):
                    es = sb.tile([P, 2 * RPC], BF, name=f"es{tl}_{hh}_{j}",
                                 tag=f"es{hh}", bufs=2)
                    nc.scalar.activation(es, sss[hh], AF.Exp, scale=0.125)
                    ess.append(es)
                for half in range(2):
                    i = 2 * j + half
                    for hh in range(2):
                        h = 2 * tl + hh
                        nc.tensor.matmul(
                            out=pos[hh],
                            lhsT=vps[i][:, h * (HDIM + 1):(h + 1) * (HDIM + 1)],
                            rhs=ess[hh][:, half * RPC:(half + 1) * RPC],
                            start=(i == 0), stop=(i == NT - 1))
            for hh in range(2):
                h = 2 * tl + hh
                bpart = HDIM * hh
                po = pos[hh]
                nc.vector.tensor_copy(out=dall[32 * h:32 * h + 1, :],
                                      in_=po[HDIM:HDIM + 1, :])
                onum = sb.tile([HDIM, RPC], F32, name=f"onum{h}",
                               tag=f"onum{h}")
                nc.vector.tensor_copy(out=onum, in_=po[0:HDIM, :])
                onums.append(onum)
        # one 4-lane reciprocal for all heads, bf16 out for the K=1 broadcast
        dinv = sb.tile([P, RPC], BF, name="dinv", tag="dinv")
        with nc.allow_low_precision(reason="1/D broadcast operand in bf16"):
            nc.vector.reciprocal(dinv, dall)
        for tl in range(2):
            for hh in range(2):
                h = 2 * tl + hh
                bpart = HDIM * hh
                pb = pp.tile([HDIM, RPC], F32, name=f"pb{h}", tag="mm", bufs=2)
                nc.tensor.matmul(out=pb, lhsT=ones_bf[32 * h:32 * h + 1, 0:HDIM],
                                 rhs=dinv[32 * h:32 * h + 1, :], start=True,
                                 stop=True, tile_position=(32 * h, 0))
                nc.vector.tensor_tensor(out=oTs[tl][bpart:bpart + HDIM, :],
                                        in0=pb, in1=onums[h],
                                        op=mybir.AluOpType.mult)

        # ---------------- Phase G: conv2 (after attention) -----------------
        for m in range(KCH_HID):
            for kc in range(NT):
                conv2_mm(m, kc)

        # ---------------- Phase H/K: x_gnn, x_proj, final ------------------
        for m in range(MT_Q):
            pg = pp.tile([P, CLS], F32, name=f"xg_ps{m}", tag="mm", bufs=2)
            for k in range(KCH_HID):
                nc.tensor.matmul(out=pg, lhsT=h2Ts[k][:, m * P:(m + 1) * P],
                                 rhs=lwa[:, k, :], start=(k == 0), stop=False)
            nc.tensor.matmul(out=pg, lhsT=ones_bf[0:1, 0:P], rhs=brows,
                             start=False, stop=True)
            xg = sb.tile([P, CLS], F32, name=f"xg{m}", tag="xg", bufs=2)
            nc.vector.tensor_copy(out=xg, in_=pg)

            pj = pp.tile([P, CLS], F32, name=f"xp_ps{m}", tag="mm", bufs=2)
            for k in range(2):
                nc.tensor.matmul(out=pj, lhsT=oTs[k][:, m * P:(m + 1) * P],
                                 rhs=Pcs[k], start=(k == 0), stop=(k == 1))
            tadd = sb.tile([P, CLS], F32, name=f"tadd{m}", tag="tadd", bufs=2)
            nc.vector.tensor_tensor(out=tadd, in0=pj, in1=xg,
                                    op=mybir.AluOpType.add)
            osb = sb.tile([P, CLS], F32, name=f"osb{m}", tag="osb", bufs=2)
            nc.scalar.activation(osb, tadd, AF.Relu)
            nc.sync.dma_start(out=out[m * P:(m + 1) * P, :], in_=osb)


_CACHE = {}


def _get_compiled():
    if "nc" not in _CACHE:
        nc = bacc.Bacc("TRN2", target_bir_lowering=False, debug=False,
                       num_devices=NC)
        with tile.TileContext(nc) as tc:
            _emit(tc)
        nc.compile()
        _CACHE["nc"] = nc
    return _CACHE["nc"]


def _prepare_in_maps(inputs):
    bf16 = ml_dtypes.bfloat16
    x = np.asarray(inputs["x"], dtype=np.float32)
    ei = np.asarray(inputs["edge_index"]).astype(np.int64)

    loop = np.arange(N_NODES, dtype=np.int64)
    src = np.concatenate([ei[0], loop])
    dst = np.concatenate([ei[1], loop])
    deg = np.bincount(dst, minlength=N_NODES).astype(np.float64)
    dinv = np.where(deg > 0, 1.0 / np.sqrt(deg), 0.0)
    normv = dinv[src] * dinv[dst]
    A = np.bincount(dst * N_NODES + src, weights=normv,
                    minlength=N_NODES * N_NODES).astype(np.float32)
    A = A.reshape(N_NODES, N_NODES)

    def pack(arr):
        # [C*P, D] -> [P, C*D]: partition-row p holds chunks c contiguously,
        # so each DMA descriptor reads long contiguous per-partition runs.
        cp, d = arr.shape
        c = cp // P
        return np.ascontiguousarray(
            arr.reshape(c, P, d).transpose(1, 0, 2).reshape(P, c * d))

    xn = pack(x.astype(bf16))
    xT = np.ascontiguousarray(x.T).astype(bf16)
    w1 = pack(np.asarray(inputs["gcn1_w"], np.float32).astype(bf16))
    w2 = pack(np.asarray(inputs["gcn2_w"], np.float32).astype(bf16))
    lwv = pack(np.asarray(inputs["lin_w"], np.float32).astype(bf16))
    ipT = np.ascontiguousarray(np.asarray(inputs["in_proj_w"], np.float32).T).astype(bf16)
    opwv = pack(np.asarray(inputs["out_proj_w"], np.float32).astype(bf16))
    pwv = pack(np.asarray(inputs["proj_w"], np.float32).astype(bf16))

    b1 = np.asarray(inputs["gcn1_b"], np.float32)
    b2 = np.asarray(inputs["gcn2_b"], np.float32)
    ipb = np.asarray(inputs["in_proj_b"], np.float32)
    opw_f = np.asarray(inputs["out_proj_w"], np.float32)
    pw_f = np.asarray(inputs["proj_w"], np.float32)
    # v-bias folds through softmax (rows sum to 1) and the projections:
    bprow = (np.asarray(inputs["lin_b"], np.float32)
             + (np.asarray(inputs["out_proj_b"], np.float32)
                + ipb[2 * IN_DIM:3 * IN_DIM] @ opw_f.T) @ pw_f
             + np.asarray(inputs["proj_b"], np.float32))
    bpk = np.zeros((P, 10), np.float32)
    bpk[:, 0:4] = b1.reshape(4, P).T
    bpk[:, 4:8] = b2.reshape(4, P).T
    bpk[:, 8:10] = ipb[0:IN_DIM].reshape(2, P).T
    browv = np.ascontiguousarray(bprow[None, :]).astype(bf16)

    in_maps = []
    for c in range(NC):
        aTc = pack(np.ascontiguousarray(A[c * RPC:(c + 1) * RPC, :].T)
                   .astype(bf16))
        in_maps.append({
            "xn": xn,
            "xT": xT,
            "xq": pack(np.ascontiguousarray(xT[:, c * RPC:(c + 1) * RPC])),
            "aT": aTc,
            "w1": w1, "w2": w2, "lw": lwv, "ipT": ipT,
            "opw": opwv, "pw": pwv,
            "bp": bpk, "brow": browv,
        })
    return in_maps


def _run(inputs, trace=False):
    nc = _get_compiled()
    in_maps = _prepare_in_maps(inputs)
    res = run_bass_kernel_spmd(nc, in_maps, core_ids=list(range(NC)),
                               trace=trace)
    out = np.concatenate([res.results[c]["out"] for c in range(NC)], axis=0)
    return np.ascontiguousarray(out.astype(np.float32)), res


def kernel(**inputs):
    out, _ = _run(inputs, trace=False)
    return out
